# revision 12
# baseline (speedup 1.0000x reference)
"""BinaryConnect 3x3 SAME conv (NHWC, 32x112x112x128 -> 32x112x112x256) on 8 trn2 cores.

Strategy (data-parallel, 4 images per core, duty-tuned fp8-DoubleRow / fp16):
  - Host: binarize kernel to +/-1 (exact). Quantize x to e4m3 (x8, rows padded
    to 128B for the DoubleRow k-tile step constraint) AND fp16 (x16, rows of
    114), both channel-major with a 1-px zero halo.
  - Device: each output tile (4 rows x 112 cols, one cout half) accumulates
    its 9 conv taps in PSUM. Groups alternate composition to keep the chip
    under its power-throttle threshold (fp8 DoubleRow draws ~2x array power;
    >~22% DR duty trips a chip-wide 2.4->2.0 GHz P-state):
      * even groups: 2 DR pair slots (taps (0,0)+(1,0), (0,1)+(1,1)) + 5 fp16
      * odd groups:  1 DR pair slot (taps (0,0)+(1,0)) + 7 fp16
    DR pairs use an overlapping 4D AP whose k-tile dim steps one image row.
    The DR slots of a block of 2-4 output-tile groups (both cout halves of a
    tile pair) are issued back-to-back before the
    block's fp16 slots: the PE pays its ~16ns DR->fp16 transition once per
    block instead of once per group.
  - Output fp16, un-transposed + cast to fp32 on host.
"""

import os

import numpy as np
import ml_dtypes

import concourse.bass as bass
import concourse.mybir as mybir
import concourse.tile as tile
from concourse import bacc
from concourse.bass_utils import run_bass_kernel_spmd

N_CORES = 8
NPC = 4            # images per core
H = 112
WP8 = 128          # fp8 padded row width (16B-aligned DoubleRow k-tile step)
WP6 = 114          # fp16 padded row width
HP = 115           # 1 top pad + 112 rows + 1 bottom pad + 1 zero tail row
CI = 128
CO = 256
TROWS = 4          # output rows per matmul tile
S = TROWS * H      # 448 matmul free dim (<=512 fp32 PSUM bank)
BROWS = 28         # output rows per input band
NB = H // BROWS    # 4 bands per image
BIN = BROWS + 3    # input rows per band incl. halo
TSB = BROWS // TROWS  # 7 tiles per band

# tap ids 0..8 in (dh,dw) raster order; fp16 weight tile indexed by tap id
TAPS = [(dh, dw) for dh in range(3) for dw in range(3)]
TID = {t: i for i, t in enumerate(TAPS)}
PAIR_SLOTS = [((0, 0), (1, 0)), ((0, 1), (1, 1))]  # DR slot s pairs these taps

F8 = ml_dtypes.float8_e4m3
DR = mybir.MatmulPerfMode.DoubleRow

_nc_cache = None
LAST_RESULT = None


def _npairs(gidx):
    return 2 if gidx % 2 == 0 else 1


def _build():
    nc = bacc.Bacc(
        "TRN2",
        target_bir_lowering=False,
        debug=False,
        num_devices=N_CORES,
    )
    x8_d = nc.dram_tensor(
        "x8", [CI, NPC, HP, WP8], mybir.dt.float8e4, kind="ExternalInput"
    )
    x6_d = nc.dram_tensor(
        "x16", [CI, NPC, HP, WP6], mybir.dt.float16, kind="ExternalInput"
    )
    w8_d = nc.dram_tensor(
        "w8", [CI, 2, 2, 2, 128], mybir.dt.float8e4, kind="ExternalInput"
    )
    w6_d = nc.dram_tensor(
        "w16", [CI, 2, 9 * 128], mybir.dt.float16, kind="ExternalInput"
    )
    o_d = nc.dram_tensor(
        "out_cm", [CO, NPC, H * H], mybir.dt.float16, kind="ExternalOutput"
    )
    with tile.TileContext(nc) as tc:
        with (
            tc.tile_pool(name="xpool", bufs=1) as xpool,
            tc.tile_pool(name="wpool", bufs=1) as wpool,
            tc.tile_pool(name="psum", bufs=8, space=bass.MemorySpace.PSUM) as psum,
            tc.tile_pool(name="opool", bufs=10) as opool,
        ):
            # Warmup operand with no DMA dependency (memset) so PE warmup can
            # start right after the framework preamble.
            wta = wpool.tile([CI, S], mybir.dt.float16, tag="wta", name="wta")
            nc.gpsimd.memset(wta[:], 0.0)
            w8t = wpool.tile([CI, 2, 2, 2, 128], mybir.dt.float8e4, tag="w8", name="w8")
            nc.sync.dma_start(w8t[:], w8_d[:])
            # Small first chunks of image 0 (rows 0-11) on the ACT ring so the
            # first matmul groups gate on small DMAs.
            xa8 = xpool.tile([CI, 12, WP8], mybir.dt.float8e4, tag="xa8", name="xa8")
            nc.scalar.dma_start(xa8[:], x8_d[:, 0, 0:12, :])
            xa6 = xpool.tile([CI, 12, WP6], mybir.dt.float16, tag="xa6", name="xa6")
            nc.scalar.dma_start(xa6[:], x6_d[:, 0, 0:12, :])
            w6t = wpool.tile([CI, 2, 9 * 128], mybir.dt.float16, tag="w6", name="w6")
            nc.sync.dma_start(w6t[:], w6_d[:])
            # PE warmup: throwaway matmuls to reach HAM K=8/8 before the real
            # stream begins.
            wu = psum.tile([128, S], mybir.dt.float32, name="ps")
            for _ in range(9):
                nc.tensor.matmul(
                    wu[:], wta[:, 0:128], wta[:, 0:S], start=True, stop=True
                )
            xs8, xs6 = {}, {}
            for b in range(NB):
                r0b = b * BROWS
                xt = xpool.tile([CI, BIN, WP8], mybir.dt.float8e4,
                                tag=f"a0_{b}", name=f"a0_{b}")
                xt6 = xpool.tile([CI, BIN, WP6], mybir.dt.float16,
                                 tag=f"b0_{b}", name=f"b0_{b}")
                if b == 0:
                    # band 0 gates early compute: split its DMAs into chunks so
                    # they spread across queues and arrive sooner.
                    for (ra, rb) in [(0, 16), (16, BIN)]:
                        nc.sync.dma_start(
                            xt[:, ra:rb, :], x8_d[:, 0, r0b + ra : r0b + rb, :]
                        )
                        nc.sync.dma_start(
                            xt6[:, ra:rb, :], x6_d[:, 0, r0b + ra : r0b + rb, :]
                        )
                else:
                    nc.sync.dma_start(xt[:], x8_d[:, 0, r0b : r0b + BIN, :])
                    nc.sync.dma_start(xt6[:], x6_d[:, 0, r0b : r0b + BIN, :])
                xs8[0, b] = xt
                xs6[0, b] = xt6
            for n in range(1, NPC):
                xt = xpool.tile([CI, HP, WP8], mybir.dt.float8e4,
                                tag=f"ai{n}", name=f"ai{n}")
                nc.sync.dma_start(xt[:], x8_d[:, n, :, :])
                for b in range(NB):
                    xs8[n, b] = xt
                xt = xpool.tile([CI, HP, WP6], mybir.dt.float16,
                                tag=f"bi{n}", name=f"bi{n}")
                nc.sync.dma_start(xt[:], x6_d[:, n, :, :])
                for b in range(NB):
                    xs6[n, b] = xt
            pairs = [(0, 1), (2, 3), (4, 5), (6,)]
            gctr = [0]

            def emit_block(n, b, worklist, ots):
                """worklist: [(j, st, half, ot)] — DR slots of every group
                first, then fp16 slots + cast per group."""
                groups = []
                for (j, st, half, ot) in worklist:
                    gidx = gctr[0]
                    gctr[0] += 1
                    npair = _npairs(gidx)
                    ps = psum.tile([128, S], mybir.dt.float32, name="ps")
                    groups.append((j, st, half, ot, npair, ps))
                early = n == 0 and b == 0 and worklist[0][1] <= 1

                def srcs(st):
                    if n == 0:
                        r0 = st * TROWS
                        if early:
                            return r0, xa8, xa6
                        return r0, xs8[n, b], xs6[n, b]
                    return b * BROWS + st * TROWS, xs8[n, b], xs6[n, b]

                for (j, st, half, ot, npair, ps) in groups:
                    r0, s8, s6 = srcs(st)
                    for si in range(npair):
                        dw = si  # pair slot si covers taps (0,si),(1,si)
                        nat = s8[:, r0 : r0 + TROWS, dw : dw + H]
                        pstep = nat.ap[0][0]
                        rhs = bass.AP(
                            nat.tensor, r0 * WP8 + dw,
                            [[pstep, CI], [WP8, 2], [WP8, TROWS], [1, H]],
                        )
                        nc.tensor.matmul(
                            ps[:], w8t[:, half, si, :, :], rhs,
                            start=(si == 0), stop=False, perf_mode=DR,
                        )
                for (j, st, half, ot, npair, ps) in groups:
                    r0, s8, s6 = srcs(st)
                    ptaps = {t for sl in PAIR_SLOTS[:npair] for t in sl}
                    f16taps = [t for t in TAPS if t not in ptaps]
                    for i, (dh, dw) in enumerate(f16taps):
                        t = TID[(dh, dw)]
                        rhs = s6[:, r0 + dh : r0 + dh + TROWS, dw : dw + H]
                        nc.tensor.matmul(
                            ps[:], w6t[:, half, t * 128 : t * 128 + 128], rhs,
                            start=False, stop=(i == len(f16taps) - 1),
                        )
                    nc.vector.tensor_copy(ot[:, j * S : (j + 1) * S], ps[:])

            def emit_dma(n, b, half, sts, ot):
                width = len(sts) * S
                o0 = (b * BROWS + sts[0] * TROWS) * H
                nc.scalar.dma_start(
                    o_d[half * 128 : half * 128 + 128, n, o0 : o0 + width],
                    ot[:, 0:width],
                )

            for n in range(NPC):
                for b in range(NB):
                    for sts in pairs:
                        if (n, b, sts) == (0, 0, (0, 1)):
                            ots = [
                                opool.tile([128, 2 * S], mybir.dt.float16, name="ot")
                                for _ in range(2)
                            ]
                            emit_block(n, b, [
                                (0, sts[0], 0, ots[0]),
                                (0, sts[0], 1, ots[1]),
                                (1, sts[1], 0, ots[0]),
                                (1, sts[1], 1, ots[1]),
                            ], ots)
                            for half in range(2):
                                emit_dma(n, b, half, sts, ots[half])
                        else:
                            ots = [
                                opool.tile([128, 2 * S], mybir.dt.float16, name="ot")
                                for _ in range(2)
                            ]
                            emit_block(n, b, [
                                (j, st, half, ots[half])
                                for half in range(2)
                                for j, st in enumerate(sts)
                            ], ots)
                            for half in range(2):
                                emit_dma(n, b, half, sts, ots[half])
    nc.compile()
    return nc


def _get_nc():
    global _nc_cache
    if _nc_cache is None:
        _nc_cache = _build()
    return _nc_cache


def kernel(x, kernel):
    global LAST_RESULT
    x = np.asarray(x)
    k = np.asarray(kernel)

    wb = np.where(k >= 0, np.float32(1), np.float32(-1))  # [3,3,128,256]
    w8 = np.zeros((CI, 2, 2, 2, 128), np.float32)
    for half in range(2):
        co = slice(half * 128, half * 128 + 128)
        for si, ((dhA, dwA), (dhB, dwB)) in enumerate(PAIR_SLOTS):
            w8[:, half, si, 0, :] = wb[dhA, dwA, :, co]
            w8[:, half, si, 1, :] = wb[dhB, dwB, :, co]
    w8 = np.ascontiguousarray(w8.astype(F8))
    w16 = np.zeros((CI, 2, 9 * 128), np.float16)
    for half in range(2):
        co = slice(half * 128, half * 128 + 128)
        for t, (dh, dw) in enumerate(TAPS):
            w16[:, half, t * 128 : t * 128 + 128] = wb[dh, dw, :, co]

    x8 = x.astype(F8)
    x16 = x.astype(np.float16)

    in_maps = []
    for c in range(N_CORES):
        sl = slice(c * NPC, (c + 1) * NPC)
        xp8 = np.zeros((CI, NPC, HP, WP8), F8)
        xp8[:, :, 1:113, 1:113] = x8[sl].transpose(3, 0, 1, 2)
        xp6 = np.zeros((CI, NPC, HP, WP6), np.float16)
        xp6[:, :, 1:113, 1:113] = x16[sl].transpose(3, 0, 1, 2)
        in_maps.append({"x8": xp8, "x16": xp6, "w8": w8, "w16": w16})

    nc = _get_nc()
    trace = os.environ.get("BCONV_TRACE", "0") == "1"
    kwargs = {}
    if trace and os.environ.get("BCONV_TRACE_CORES", "") == "all":
        kwargs["trace_cores"] = list(range(N_CORES))
    res = run_bass_kernel_spmd(
        nc, in_maps, core_ids=list(range(N_CORES)), trace=trace, **kwargs
    )
    LAST_RESULT = res

    out = np.empty((32, H, H, CO), np.float32)
    for c in range(N_CORES):
        o = res.results[c]["out_cm"].reshape(CO, NPC, H, H).astype(np.float32)
        out[c * NPC : (c + 1) * NPC] = o.transpose(1, 2, 3, 0)
    return out


# revision 13
# speedup vs baseline: 1.1854x; 1.1854x over previous
"""BinaryConnect 3x3 SAME conv (NHWC, 32x112x112x128 -> 32x112x112x256) on 8 trn2 cores.

Strategy (data-parallel, 4 images per core, duty-tuned fp8-DoubleRow / fp16):
  - Host: binarize kernel to +/-1 (exact). Quantize x to e4m3 (x8, rows padded
    to 128B for the DoubleRow k-tile step constraint) AND fp16 (x16, rows of
    114), both channel-major with a 1-px zero halo.
  - Device: each output tile (4 rows x 112 cols, one cout half) accumulates
    its 9 conv taps in PSUM. Groups alternate composition to keep the chip
    under its power-throttle threshold (fp8 DoubleRow draws ~2x array power;
    >~22% DR duty trips a chip-wide 2.4->2.0 GHz P-state):
      * even groups: 2 DR pair slots (taps (0,0)+(1,0), (0,1)+(1,1)) + 5 fp16
      * odd groups:  1 DR pair slot (taps (0,0)+(1,0)) + 7 fp16
    DR pairs use an overlapping 4D AP whose k-tile dim steps one image row.
    The DR slots of a block of 2-4 groups are issued back-to-back before the
    block's fp16 slots: the PE pays its ~16ns DR->fp16 transition once per
    block instead of once per group.
  - Output fp16, un-transposed + cast to fp32 on host.
"""

import os

import numpy as np
import ml_dtypes

import concourse.bass as bass
import concourse.mybir as mybir
import concourse.tile as tile
from concourse import bacc
from concourse.bass_utils import run_bass_kernel_spmd

N_CORES = 8
NPC = 4            # images per core
H = 112
WP8 = 128          # fp8 padded row width (16B-aligned DoubleRow k-tile step)
WP6 = 114          # fp16 padded row width
HP = 115           # 1 top pad + 112 rows + 1 bottom pad + 1 zero tail row
CI = 128
CO = 256
TROWS = 4          # output rows per matmul tile
S = TROWS * H      # 448 matmul free dim (<=512 fp32 PSUM bank)
BROWS = 28         # output rows per input band
NB = H // BROWS    # 4 bands per image
BIN = BROWS + 3    # input rows per band incl. halo
TSB = BROWS // TROWS  # 7 tiles per band

# tap ids 0..8 in (dh,dw) raster order; fp16 weight tile indexed by tap id
TAPS = [(dh, dw) for dh in range(3) for dw in range(3)]
TID = {t: i for i, t in enumerate(TAPS)}
PAIR_SLOTS = [((0, 0), (1, 0)), ((0, 1), (1, 1))]  # DR slot s pairs these taps

F8 = ml_dtypes.float8_e4m3
DR = mybir.MatmulPerfMode.DoubleRow

_nc_cache = None
LAST_RESULT = None


def _npairs(gidx):
    return 2 if gidx % 2 == 0 else 1


def _build():
    nc = bacc.Bacc(
        "TRN2",
        target_bir_lowering=False,
        debug=False,
        num_devices=N_CORES,
    )
    x8_d = nc.dram_tensor(
        "x8", [CI, NPC, HP, WP8], mybir.dt.float8e4, kind="ExternalInput"
    )
    x6_d = nc.dram_tensor(
        "x16", [CI, NPC, HP, WP6], mybir.dt.float16, kind="ExternalInput"
    )
    w8_d = nc.dram_tensor(
        "w8", [CI, 2, 2, 2, 128], mybir.dt.float8e4, kind="ExternalInput"
    )
    w6_d = nc.dram_tensor(
        "w16", [CI, 2, 9 * 128], mybir.dt.float16, kind="ExternalInput"
    )
    o_d = nc.dram_tensor(
        "out_cm", [CO, NPC, H * H], mybir.dt.float16, kind="ExternalOutput"
    )
    with tile.TileContext(nc) as tc:
        with (
            tc.tile_pool(name="xpool", bufs=1) as xpool,
            tc.tile_pool(name="wpool", bufs=1) as wpool,
            tc.tile_pool(name="psum", bufs=8, space=bass.MemorySpace.PSUM) as psum,
            tc.tile_pool(name="opool", bufs=10) as opool,
        ):
            # Warmup operand with no DMA dependency (memset) so PE warmup can
            # start right after the framework preamble.
            wta = wpool.tile([CI, S], mybir.dt.float16, tag="wta", name="wta")
            nc.gpsimd.memset(wta[:], 0.0)
            w8t = wpool.tile([CI, 2, 2, 2, 128], mybir.dt.float8e4, tag="w8", name="w8")
            nc.sync.dma_start(w8t[:], w8_d[:])
            # Small first chunks of image 0 (rows 0-11) on the ACT ring so the
            # first matmul groups gate on small DMAs.
            xa8 = xpool.tile([CI, 12, WP8], mybir.dt.float8e4, tag="xa8", name="xa8")
            nc.scalar.dma_start(xa8[:], x8_d[:, 0, 0:12, :])
            xa6 = xpool.tile([CI, 12, WP6], mybir.dt.float16, tag="xa6", name="xa6")
            nc.scalar.dma_start(xa6[:], x6_d[:, 0, 0:12, :])
            w6t = wpool.tile([CI, 2, 9 * 128], mybir.dt.float16, tag="w6", name="w6")
            nc.sync.dma_start(w6t[:], w6_d[:])
            # PE warmup: throwaway matmuls to reach HAM K=8/8 before the real
            # stream begins.
            wu = psum.tile([128, S], mybir.dt.float32, name="ps")
            for _ in range(9):
                nc.tensor.matmul(
                    wu[:], wta[:, 0:128], wta[:, 0:S], start=True, stop=True
                )
            xs8, xs6 = {}, {}
            for b in range(NB):
                r0b = b * BROWS
                xt = xpool.tile([CI, BIN, WP8], mybir.dt.float8e4,
                                tag=f"a0_{b}", name=f"a0_{b}")
                xt6 = xpool.tile([CI, BIN, WP6], mybir.dt.float16,
                                 tag=f"b0_{b}", name=f"b0_{b}")
                if b == 0:
                    # band 0 gates early compute: split its DMAs into chunks so
                    # they spread across queues and arrive sooner.
                    for (ra, rb) in [(0, 16), (16, BIN)]:
                        nc.sync.dma_start(
                            xt[:, ra:rb, :], x8_d[:, 0, r0b + ra : r0b + rb, :]
                        )
                        nc.sync.dma_start(
                            xt6[:, ra:rb, :], x6_d[:, 0, r0b + ra : r0b + rb, :]
                        )
                else:
                    nc.sync.dma_start(xt[:], x8_d[:, 0, r0b : r0b + BIN, :])
                    nc.sync.dma_start(xt6[:], x6_d[:, 0, r0b : r0b + BIN, :])
                xs8[0, b] = xt
                xs6[0, b] = xt6
            for n in range(1, NPC):
                xt = xpool.tile([CI, HP, WP8], mybir.dt.float8e4,
                                tag=f"ai{n}", name=f"ai{n}")
                nc.sync.dma_start(xt[:], x8_d[:, n, :, :])
                for b in range(NB):
                    xs8[n, b] = xt
                xt = xpool.tile([CI, HP, WP6], mybir.dt.float16,
                                tag=f"bi{n}", name=f"bi{n}")
                nc.sync.dma_start(xt[:], x6_d[:, n, :, :])
                for b in range(NB):
                    xs6[n, b] = xt
            pairs = [(0, 1), (2, 3), (4, 5), (6,)]
            gctr = [0]

            def emit_block(n, b, worklist, ots):
                """worklist: [(j, st, half, ot)] — DR slots of every group
                first, then fp16 slots + cast per group."""
                groups = []
                for (j, st, half, ot) in worklist:
                    gidx = gctr[0]
                    gctr[0] += 1
                    npair = _npairs(gidx)
                    ps = psum.tile([128, S], mybir.dt.float32, name="ps")
                    groups.append((j, st, half, ot, npair, ps))
                early = n == 0 and b == 0 and worklist[0][1] <= 1

                def srcs(st):
                    if n == 0:
                        r0 = st * TROWS
                        if early:
                            return r0, xa8, xa6
                        return r0, xs8[n, b], xs6[n, b]
                    return b * BROWS + st * TROWS, xs8[n, b], xs6[n, b]

                for (j, st, half, ot, npair, ps) in groups:
                    r0, s8, s6 = srcs(st)
                    for si in range(npair):
                        dw = si  # pair slot si covers taps (0,si),(1,si)
                        nat = s8[:, r0 : r0 + TROWS, dw : dw + H]
                        pstep = nat.ap[0][0]
                        rhs = bass.AP(
                            nat.tensor, r0 * WP8 + dw,
                            [[pstep, CI], [WP8, 2], [WP8, TROWS], [1, H]],
                        )
                        nc.tensor.matmul(
                            ps[:], w8t[:, half, si, :, :], rhs,
                            start=(si == 0), stop=False, perf_mode=DR,
                        )
                for (j, st, half, ot, npair, ps) in groups:
                    r0, s8, s6 = srcs(st)
                    ptaps = {t for sl in PAIR_SLOTS[:npair] for t in sl}
                    f16taps = [t for t in TAPS if t not in ptaps]
                    for i, (dh, dw) in enumerate(f16taps):
                        t = TID[(dh, dw)]
                        rhs = s6[:, r0 + dh : r0 + dh + TROWS, dw : dw + H]
                        nc.tensor.matmul(
                            ps[:], w6t[:, half, t * 128 : t * 128 + 128], rhs,
                            start=False, stop=(i == len(f16taps) - 1),
                        )
                    nc.vector.tensor_copy(ot[:, j * S : (j + 1) * S], ps[:])

            def emit_dma(n, b, half, sts, ot):
                width = len(sts) * S
                o0 = (b * BROWS + sts[0] * TROWS) * H
                nc.scalar.dma_start(
                    o_d[half * 128 : half * 128 + 128, n, o0 : o0 + width],
                    ot[:, 0:width],
                )

            for n in range(NPC):
                for b in range(NB):
                    for sts in pairs:
                        if (n, b, sts) == (0, 0, (0, 1)):
                            ots = [
                                opool.tile([128, 2 * S], mybir.dt.float16, name="ot")
                                for _ in range(2)
                            ]
                            emit_block(n, b, [
                                (0, sts[0], 0, ots[0]),
                                (0, sts[0], 1, ots[1]),
                                (1, sts[1], 0, ots[0]),
                                (1, sts[1], 1, ots[1]),
                            ], ots)
                            for half in range(2):
                                emit_dma(n, b, half, sts, ots[half])
                        else:
                            for half in range(2):
                                ot = opool.tile(
                                    [128, 2 * S], mybir.dt.float16, name="ot"
                                )
                                emit_block(n, b, [
                                    (j, st, half, ot) for j, st in enumerate(sts)
                                ], [ot])
                                emit_dma(n, b, half, sts, ot)
    nc.compile()
    return nc


def _get_nc():
    global _nc_cache
    if _nc_cache is None:
        _nc_cache = _build()
    return _nc_cache


def kernel(x, kernel):
    global LAST_RESULT
    x = np.asarray(x)
    k = np.asarray(kernel)

    wb = np.where(k >= 0, np.float32(1), np.float32(-1))  # [3,3,128,256]
    w8 = np.zeros((CI, 2, 2, 2, 128), np.float32)
    for half in range(2):
        co = slice(half * 128, half * 128 + 128)
        for si, ((dhA, dwA), (dhB, dwB)) in enumerate(PAIR_SLOTS):
            w8[:, half, si, 0, :] = wb[dhA, dwA, :, co]
            w8[:, half, si, 1, :] = wb[dhB, dwB, :, co]
    w8 = np.ascontiguousarray(w8.astype(F8))
    w16 = np.zeros((CI, 2, 9 * 128), np.float16)
    for half in range(2):
        co = slice(half * 128, half * 128 + 128)
        for t, (dh, dw) in enumerate(TAPS):
            w16[:, half, t * 128 : t * 128 + 128] = wb[dh, dw, :, co]

    x8 = x.astype(F8)
    x16 = x.astype(np.float16)

    in_maps = []
    for c in range(N_CORES):
        sl = slice(c * NPC, (c + 1) * NPC)
        xp8 = np.zeros((CI, NPC, HP, WP8), F8)
        xp8[:, :, 1:113, 1:113] = x8[sl].transpose(3, 0, 1, 2)
        xp6 = np.zeros((CI, NPC, HP, WP6), np.float16)
        xp6[:, :, 1:113, 1:113] = x16[sl].transpose(3, 0, 1, 2)
        in_maps.append({"x8": xp8, "x16": xp6, "w8": w8, "w16": w16})

    nc = _get_nc()
    trace = os.environ.get("BCONV_TRACE", "0") == "1"
    kwargs = {}
    if trace and os.environ.get("BCONV_TRACE_CORES", "") == "all":
        kwargs["trace_cores"] = list(range(N_CORES))
    res = run_bass_kernel_spmd(
        nc, in_maps, core_ids=list(range(N_CORES)), trace=trace, **kwargs
    )
    LAST_RESULT = res

    out = np.empty((32, H, H, CO), np.float32)
    for c in range(N_CORES):
        o = res.results[c]["out_cm"].reshape(CO, NPC, H, H).astype(np.float32)
        out[c * NPC : (c + 1) * NPC] = o.transpose(1, 2, 3, 0)
    return out
